# revision 25
# baseline (speedup 1.0000x reference)
"""Distributed Trainium2 kernel for nn_DecoderAttentionRotary (v2).

Strategy (8 NeuronCores, tensor-parallel over heads, fp16 matmul datapath):
  - host: transpose x -> xT [D, B*L] fp16; per-core Wqkv column slice
    reordered to [q0,k0,q1,k1,v0|v1] fp16; cos/sin transposed f32;
    causal masks fp16.
  - device, per core (2 heads):
      phase 1 b0: qkT = (Wqk^T @ xT) + b (fp16 matmuls, fp32 psum),
               v = x @ Wv in [l, hd] layout; RoPE fused per 512-col chunk.
      phase B: phase-1 b1 chunks interleaved (by emission priority) with
               attention blocks for b0 -> ACT exp and DVE work hide under
               the dense QKV matmul stream; per-head AllToAll for b0
               issued mid-phase.
      phase C: attention b1 (h0 then h1), per-head A2A; proj for b0 rows
               overlaps the A2A / attention tail.
      phase D: proj for b1 rows; Wd streamed once (n4-outer, both row
               halves per n4 tile).
  - attention in scores^T layout: scoresT[k,q] -> exp (ACT, fp16) ->
    mask (DVE) -> out^T accum + ones-matmul rowsums (PE) -> normalize via
    reciprocal_approx_fast + a PE ones-outer-product broadcast.
  - host: scatter the per-core 256-row halves into the full output.
"""
import sys

for _p in ("/opt/pypackages", "/opt/trn_rl_repo"):
    if _p not in sys.path:
        sys.path.insert(0, _p)

import numpy as np

B, L, D, H = 2, 2048, 2048, 16
HD, R = 128, 32
SCALE = float(HD) ** -0.5
W = 8
HPC = H // W              # heads per core
M = B * L                 # flattened rows
CORES = list(range(W))

_NC = None


def _build_nc():
    import concourse.mybir as mybir
    import concourse.tile as tile
    from concourse import bacc

    f32 = mybir.dt.float32
    f16 = mybir.dt.float16
    AFT = mybir.ActivationFunctionType
    OP = mybir.AluOpType

    nc = bacc.Bacc(None, target_bir_lowering=False, num_devices=W)
    xT = nc.declare_dram_parameter("xT", [D, M], f16, isOutput=False)
    wqkv = nc.declare_dram_parameter("wqkv", [D, 6 * HD], f16, isOutput=False)
    bqk = nc.declare_dram_parameter("bqk", [4 * HD, 1], f32, isOutput=False)
    bvb = nc.declare_dram_parameter("bvb", [128, 2 * HD], f16, isOutput=False)
    cosT = nc.declare_dram_parameter("cosT", [R, L], f16, isOutput=False)
    sinT = nc.declare_dram_parameter("sinT", [R, L], f16, isOutput=False)
    masks = nc.declare_dram_parameter("masks", [4, 128, 512], f16, isOutput=False)
    wd = nc.declare_dram_parameter("wd", [D, D], f16, isOutput=False)
    bdb = nc.declare_dram_parameter("bdb", [128, D], f32, isOutput=False)
    onesc = nc.declare_dram_parameter("onesc", [128, 1], f16, isOutput=False)
    y = nc.declare_dram_parameter("y", [M // W, D], f16, isOutput=True)

    xT_r = xT.ap().rearrange("(t p) n -> p t n", p=128)   # [128, 16, M]
    wd_r = wd.ap().rearrange("(t p) n -> p t n", p=128)

    with tile.TileContext(nc) as tc:
        with (
            tc.tile_pool(name="const", bufs=1) as cpool,
            tc.tile_pool(name="dram", bufs=1, space="DRAM") as dpool,
            tc.tile_pool(name="ps", bufs=1, space="PSUM") as pp,
            tc.tile_pool(name="qkv", bufs=1) as qkvpool,
            tc.tile_pool(name="p3", bufs=1) as p3pool,      # low in stack: prefetchable
            tc.tile_pool(name="p3s", bufs=3) as p3s,
            tc.tile_pool(name="att", bufs=3) as apool,
            tc.tile_pool(name="p1", bufs=3) as p1pool,
        ):
            a2a_ins = [dpool.tile([W, 3 * HD, 256], f16, name="a2ain0"),
                       dpool.tile([W, HD, 256], f16, name="a2ain1")]
            a2a_outs = [dpool.tile([W, 3 * HD, 256], f16, name="a2aout0"),
                        dpool.tile([W, HD, 256], f16, name="a2aout1")]

            # first x chunk before weights: both feed the first mp-group,
            # and x lands on a different queue than the w stream
            w_sb = cpool.tile([128, 16, 6 * HD], f16)
            wq_r = wqkv.ap().rearrange("(t p) m -> p t m", p=128)
            xt0_tiles = []
            for half in range(2):
                xt = p1pool.tile([128, 8, 512], f16, tag="xt", bufs=3)
                for qtr in range(2):
                    nc.sync.dma_start(
                        out=xt[:, 4 * qtr:4 * (qtr + 1), :],
                        in_=xT_r[:, half * 8 + 4 * qtr:half * 8 + 4 * (qtr + 1),
                                 0:512],
                    )
                xt0_tiles.append(xt)
            for wq in range(8):
                nc.sync.dma_start(
                    out=w_sb[:, 2 * wq:2 * (wq + 1), :],
                    in_=wq_r[:, 2 * wq:2 * (wq + 1), :],
                )
            bqk_sb = cpool.tile([128, 4], f32)
            nc.sync.dma_start(
                out=bqk_sb[:], in_=bqk.ap().rearrange("(t p) o -> p (t o)", p=128)
            )
            bvb_sb = cpool.tile([128, 2 * HD], f16)
            nc.sync.dma_start(out=bvb_sb[:], in_=bvb.ap())
            ones_r = cpool.tile([1, 128], f16)
            nc.vector.memset(ones_r[:], 1.0)
            ones_c = cpool.tile([128, 1], f16)
            nc.sync.dma_start(out=ones_c[:], in_=onesc.ap())
            cos_sb = cpool.tile([R, L], f16)
            sin_sb = cpool.tile([R, L], f16)
            mask_sb = cpool.tile([128, 512], f16)
            consts_loaded = False

            qk_sbs, v_sbs, o_sbs = [], [], []
            for b in range(B):
                qk_sbs.append(qkvpool.tile([128, 4, L], f16, name=f"qk{b}"))
                v_sbs.append(qkvpool.tile([128, 16, 2 * HD], f16, name=f"v{b}"))

            # ---- phase 1 worker (one 512-token chunk of batch b) ----
            def p1_chunk(b, nch):
                nonlocal consts_loaded
                qk_sb, v_sb = qk_sbs[b], v_sbs[b]
                n0 = b * L + nch * 512
                ch = slice(nch * 512, (nch + 1) * 512)
                if b == 0 and nch == 0:
                    xt_tiles = xt0_tiles
                else:
                    xt_tiles = []
                    for half in range(2):
                        xt = p1pool.tile([128, 8, 512], f16, tag="xt", bufs=3)
                        nc.sync.dma_start(
                            out=xt[:],
                            in_=xT_r[:, half * 8:(half + 1) * 8, n0:n0 + 512],
                        )
                        xt_tiles.append(xt)
                if not consts_loaded:
                    # emitted after first xt DMAs: lower queue priority
                    nc.sync.dma_start(out=cos_sb[:], in_=cosT.ap())
                    nc.sync.dma_start(out=sin_sb[:], in_=sinT.ap())
                    nc.sync.dma_start(out=mask_sb[:], in_=masks.ap()[0])
                    consts_loaded = True
                for mp in range(2):
                    pss = [
                        pp.tile([128, 512], f32, tag="work", bufs=3,
                                name=f"qkps{b}_{nch}_{2 * mp + i}")
                        for i in range(2)
                    ]
                    for kt in range(16):
                        xt = xt_tiles[kt // 8]
                        for i in range(2):
                            m = 2 * mp + i
                            nc.tensor.matmul(
                                pss[i][:],
                                lhsT=w_sb[:, kt, m * 128:(m + 1) * 128],
                                rhs=xt[:, kt % 8, :],
                                start=(kt == 0),
                                stop=(kt == 15),
                            )
                    for i in range(2):
                        m = 2 * mp + i
                        nc.vector.tensor_scalar_add(
                            qk_sb[:, m, ch], pss[i][:], bqk_sb[:, m:m + 1]
                        )
                for m in range(4):
                    # fused RoPE on rows 0:R of this chunk
                    cs = cos_sb[:, ch]
                    sn = sin_sb[:, ch]
                    ta = p1pool.tile([R, 512], f32, tag="ta", bufs=2)
                    rot = p1pool.tile([R, 512], f16, tag="rot", bufs=2)
                    tb = p1pool.tile([R, 512], f32, tag="tb", bufs=2)
                    nc.sync.dma_start(out=rot[0:16, :], in_=qk_sb[16:32, m, ch])
                    nc.sync.dma_start(out=rot[16:32, :], in_=qk_sb[0:16, m, ch])
                    nc.vector.tensor_tensor(
                        ta[:], qk_sb[0:R, m, ch], cs, op=OP.mult
                    )
                    nc.vector.tensor_tensor(tb[:], rot[:], sn, op=OP.mult)
                    nc.vector.tensor_tensor(
                        qk_sb[0:R, m, ch], ta[:], tb[:], op=OP.add
                    )
                for rr2 in range(2):
                    vpss = [
                        pp.tile([128, 2 * HD], f32, tag="acc", bufs=3,
                                name=f"vps{b}_{nch}_{2 * rr2 + i}")
                        for i in range(2)
                    ]
                    for kt in range(16):
                        xt = xt_tiles[kt // 8]
                        for i in range(2):
                            rr = 2 * rr2 + i
                            nc.tensor.matmul(
                                vpss[i][:],
                                lhsT=xt[:, kt % 8, rr * 128:(rr + 1) * 128],
                                rhs=w_sb[:, kt, 4 * HD:6 * HD],
                                start=(kt == 0),
                                stop=(kt == 15),
                            )
                    for i in range(2):
                        rr = 2 * rr2 + i
                        # bias folded into the psum->sbuf drain on DVE
                        nc.vector.tensor_tensor(
                            v_sb[:, nch * 4 + rr, :], vpss[i][:], bvb_sb[:],
                            op=OP.add,
                        )

            # ---- attention worker (one (b, h, qc) block) ----
            def attn_block(b, h, qc):
                qk_sb, v_sb = qk_sbs[b], v_sbs[b]
                nk = 4 * qc + 4
                outp = pp.tile([128, 512], f32, tag="acc", bufs=3,
                               name=f"outp{b}_{h}_{qc}")
                sump = pp.tile([1, 512], f32, tag="sump", bufs=2,
                               name=f"sump{b}_{h}_{qc}")
                for ki in range(nk):
                    # causal: for diagonal k-tiles only q-cols >= j*128
                    j = max(0, ki - qc * 4)
                    c0 = j * 128
                    npr = 512 - c0
                    qs = slice(qc * 512 + c0, (qc + 1) * 512)
                    sp = pp.tile([128, 512], f32, tag="work", bufs=3,
                                 name=f"sp{b}_{h}_{qc}_{ki}")
                    nc.tensor.matmul(
                        sp[:, 0:npr],
                        lhsT=qk_sb[:, 2 * h + 1, ki * 128:(ki + 1) * 128],
                        rhs=qk_sb[:, 2 * h, qs],
                        start=True, stop=True,
                    )
                    et = apool.tile([128, 512], f16, tag="et", bufs=4)
                    nc.scalar.activation(
                        et[:, 0:npr], sp[:, 0:npr], AFT.Exp, scale=SCALE
                    )
                    if ki >= qc * 4:
                        nc.vector.tensor_tensor(
                            et[:, 0:128], et[:, 0:128],
                            mask_sb[:, 0:128], op=OP.mult,
                        )
                    nc.tensor.matmul(
                        outp[:, c0:512],
                        lhsT=v_sb[:, ki, h * 128:(h + 1) * 128],
                        rhs=et[:, 0:npr],
                        start=(ki == 0), stop=(ki == nk - 1),
                    )
                    nc.tensor.matmul(
                        sump[:, c0:512], lhsT=ones_c[:], rhs=et[:, 0:npr],
                        start=(ki == 0), stop=(ki == nk - 1),
                    )
                return (b, h, qc, outp, sump)

            def attn_norm(st):
                # normalize runs one block late so the PE broadcast matmul
                # never head-of-line blocks behind the reciprocal chain
                b, h, qc, outp, sump = st
                osum = apool.tile([128, 512], f32, tag="osum", bufs=3)
                nc.vector.tensor_copy(osum[:], outp[:])
                rec = apool.tile([1, 512], f32, tag="rec", bufs=2)
                nc.vector.reciprocal_approx_fast(rec[:], sump[:])
                rec16 = apool.tile([1, 512], f16, tag="rec16", bufs=2)
                nc.vector.tensor_copy(rec16[:], rec[:])
                bcs = pp.tile([128, 512], f32, tag="acc", bufs=3,
                              name=f"bcs{b}_{h}_{qc}")
                nc.tensor.matmul(
                    bcs[:], lhsT=ones_r[:], rhs=rec16[:], start=True, stop=True
                )
                ot = apool.tile([128, 512], f16, tag="ot", bufs=3)
                nc.vector.tensor_tensor(ot[:], osum[:], bcs[:], op=OP.mult)
                t, r0 = (0, (2 * b + h) * 128) if (b, h) != (1, 1) else (1, 0)
                nc.sync.dma_start(
                    out=a2a_ins[t][2 * qc:2 * qc + 2,
                                   r0:r0 + 128, :].rearrange("u p n -> p u n"),
                    in_=ot[:].rearrange("p (u n) -> p u n", u=2),
                )

            def a2a(b):
                nc.gpsimd.collective_compute(
                    "AllToAll",
                    mybir.AluOpType.bypass,
                    replica_groups=[CORES],
                    ins=[a2a_ins[b][:]],
                    outs=[a2a_outs[b][:]],
                )

            def o_piece(b, u, t, r0, eng):
                o_sb_v = o_sbs[b][:].rearrange("p (j u) n -> p j u n", j=W)
                eng.dma_start(
                    out=o_sb_v[:, :, u, :],
                    in_=a2a_outs[t][:, r0:r0 + 128, :].rearrange(
                        "j p n -> p j n"),
                )

            # ---- emission schedule ----
            # phase A: full QKV for b0
            for nch in range(4):
                p1_chunk(0, nch)

            for b in range(B):
                o_sbs.append(p3pool.tile([128, 16, 256], f16, name=f"osb{b}"))
            bd_sb = p3pool.tile([128, D], f32)
            nc.sync.dma_start(out=bd_sb[:], in_=bdb.ap())

            # phase B: QKV for b1
            for nch in range(4):
                p1_chunk(1, nch)

            # phase C/D: attention per batch, A2A after each batch.
            # o_load(0) is emitted late so its cc-completion wait does not
            # head-of-line block batch-1's a2a_in writes on the sync queue.
            pend = None
            for (b, h) in [(0, 0), (0, 1), (1, 0)]:
                for qc in reversed(range(L // 512)):
                    st = attn_block(b, h, qc)
                    if pend is not None:
                        attn_norm(pend)
                    pend = st
            attn_norm(pend)
            a2a(0)
            pend = None
            for qc in reversed(range(L // 512)):
                st = attn_block(1, 1, qc)
                if pend is not None:
                    attn_norm(pend)
                pend = st
            attn_norm(pend)
            # b0 rows + b1's h0 half arrive with the first collective
            o_piece(0, 0, 0, 0, nc.sync)
            o_piece(0, 1, 0, 128, nc.sync)
            o_piece(1, 0, 0, 256, nc.sync)
            a2a(1)

            # phases C/D tail: output projection. b0 rows fully first (they
            # only need the first A2A) so the second A2A hides under them;
            # Wd streams twice through a rotating 2-slot tag.
            for (n4, bh) in [(0, 0), (1, 0), (2, 0), (3, 0),
                             (0, 1), (1, 1), (2, 1), (3, 1)]:
                whs = []
                for kh in range(2):
                    wh = p3s.tile([128, 8, 512], f16, tag="wdt", bufs=4,
                                  name=f"wdt{n4}_{bh}_{kh}")
                    nc.sync.dma_start(
                        out=wh[:],
                        in_=wd_r[:, 8 * kh:8 * (kh + 1),
                                 n4 * 512:(n4 + 1) * 512],
                    )
                    whs.append(wh)
                if (n4, bh) == (0, 1):
                    o_piece(1, 1, 1, 0, nc.gpsimd)
                if True:
                    for i in range(2):
                        m = 2 * bh + i
                        yp = pp.tile([128, 512], f32,
                                     tag=("work" if i else "acc"), bufs=3,
                                     name=f"yps{n4}_{m}")
                        for kt in range(16):
                            nc.tensor.matmul(
                                yp[:],
                                lhsT=o_sbs[bh][:, kt, i * 128:(i + 1) * 128],
                                rhs=whs[kt // 8][:, kt % 8, :],
                                start=(kt == 0), stop=(kt == 15),
                            )
                        yt = p3s.tile([128, 512], f16, tag="yt")
                        nc.vector.tensor_tensor(
                            yt[:], yp[:], bd_sb[:, n4 * 512:(n4 + 1) * 512],
                            op=OP.add,
                        )
                        nc.sync.dma_start(
                            out=y[m * 128:(m + 1) * 128,
                                  n4 * 512:(n4 + 1) * 512],
                            in_=yt[:],
                        )
    nc.finalize()
    return nc


def _host_prep(x_BLD, cos, sin, Wqkv, bqkv, Wd, bd):
    x = np.asarray(x_BLD, np.float32).reshape(M, D)
    xT = np.ascontiguousarray(x.T.astype(np.float16))
    c2 = np.asarray(cos, np.float32).reshape(L, R).T
    s2 = np.asarray(sin, np.float32).reshape(L, R).T
    cosT = np.ascontiguousarray(c2)
    sinT_pm = np.ascontiguousarray(
        np.concatenate([-s2[:16], s2[16:]], axis=0)
    )
    kk = np.arange(128, dtype=np.int64)[:, None]
    qq = np.arange(512, dtype=np.int64)[None, :]
    masks = np.stack(
        [(qq >= j * 128 + kk).astype(np.float16) for j in range(4)]
    )
    bdb = np.ascontiguousarray(
        np.broadcast_to(np.asarray(bd, np.float32), (128, D))
    )
    Wqkv = np.asarray(Wqkv, np.float32)
    bqkv = np.asarray(bqkv, np.float32)
    in_maps = []
    for c in range(W):
        base = c * HPC * 3 * HD
        qk_idx = np.concatenate(
            [np.arange(base + h * 3 * HD, base + h * 3 * HD + 2 * HD)
             for h in range(HPC)]
        )
        v_idx = np.concatenate(
            [np.arange(base + h * 3 * HD + 2 * HD, base + (h + 1) * 3 * HD)
             for h in range(HPC)]
        )
        in_maps.append({
            "xT": xT,
            "wqkv": np.ascontiguousarray(
                Wqkv[:, np.concatenate([qk_idx, v_idx])].astype(np.float16)
            ),
            "bqk": np.ascontiguousarray(bqkv[qk_idx].reshape(4 * HD, 1)),
            "bvb": np.ascontiguousarray(np.broadcast_to(
                bqkv[v_idx].reshape(1, 2 * HD).astype(np.float16),
                (128, 2 * HD),
            )),
            "cosT": cosT.astype(np.float16),
            "sinT": sinT_pm.astype(np.float16),
            "masks": masks,
            "wd": np.asarray(Wd, np.float32).astype(np.float16),
            "bdb": bdb,
            "onesc": np.ones((128, 1), np.float16),
        })
    return in_maps


def _get_nc():
    global _NC
    if _NC is None:
        _NC = _build_nc()
    return _NC


def _run(inputs, trace=False, tmpdir=None):
    from concourse.bass_utils import run_bass_kernel_spmd

    in_maps = _host_prep(**inputs)
    nc = _get_nc()
    res = run_bass_kernel_spmd(nc, in_maps, CORES, trace=trace, tmpdir=tmpdir)
    out = np.empty((M, D), np.float32)
    for c in CORES:
        yc = res.results[c]["y"].astype(np.float32)   # [512, D]: b0 | b1
        out[c * 256:(c + 1) * 256] = yc[:256]
        out[L + c * 256:L + (c + 1) * 256] = yc[256:]
    return out.reshape(B, L, D), res


def kernel(**inputs) -> np.ndarray:
    out, _ = _run(inputs)
    return out
